# revision 2
# baseline (speedup 1.0000x reference)
"""DBRX-experts MoE kernel for 8 Trainium2 NeuronCores (expert-parallel).

Strategy
--------
E=8 experts map 1:1 onto the 8 cores. The host gathers each expert's routed
tokens (top-k dispatch done in numpy — the "all-to-all" of the sharding hint
collapses to a host-side gather because kernel() already owns the full
inputs), pads them to a common count, and pre-lays-out the expert's weights
so the device kernel is a pure dense transposed MLP:

    G^T = Wg^T-tiles @ X^T      (contract H, out [F, T])
    U^T = Wu^T-tiles @ X^T
    Hmid^T = sigmoid(G^T) * G^T * U^T        (silu(g) * u)
    Y^T = Wd^T-tiles @ Hmid^T   (contract F, out [H, T])

All matmuls keep the weights stationary ([128,128] tiles) and the tokens
moving ([128, <=512]). Weights/activations are fp16 by default: the PE runs
16-bit matmuls at 1 cycle/row at ANY moving width (fp32r needs >=256), so
the token dimension is padded only to the max per-expert count (497 for the
reference routing) instead of 512, and weight DMA traffic halves vs fp32 —
which un-saturates the DMA engines that otherwise rate-match the PE and
stall it. fp16 keeps the end-to-end relative error ~1e-3 (PE upconverts
fp16 to e10m11 exactly; accumulation is fp32 PSUM). The per-token combine
weights and the scatter-add back into the [T, H] output (the "all-reduce")
are applied on the host in fp32.

No device collectives are needed: cores are fully independent.
"""

import os

# The axon jax platform must stay visible even if the caller pinned cpu for
# its own reference computation (bass2jax needs jax.devices() -> axon).
if os.environ.get("JAX_PLATFORMS") == "cpu":
    os.environ["JAX_PLATFORMS"] = ""

import numpy as np

import concourse.bass as bass
import concourse.mybir as mybir
import concourse.tile as tile
from concourse.bass_utils import run_bass_kernel_spmd

E, H, F, P = 8, 2048, 2048, 128
HO, FO = H // P, F // P  # 16, 16

F32 = mybir.dt.float32
F32R = mybir.dt.float32r
BF16 = mybir.dt.bfloat16
FP16 = mybir.dt.float16

_DT = {"f32r": F32R, "bf16": BF16, "fp16": FP16}

_prog_cache: dict = {}


def _chunks_for(n_pad: int, dt: str):
    """Split [0, n_pad) into equal chunks of <=512 (one PSUM bank of fp32).

    For fp32r, chunks must be >=256 wide to run at 1 cycle/row (the
    _pad_count legacy path guarantees that). 16-bit dtypes run at full rate
    at any width, so chunks are exactly n_pad/n_ch.
    """
    n_ch = -(-n_pad // 512)
    assert n_pad % n_ch == 0
    cn = n_pad // n_ch
    if dt == "f32r":
        assert cn >= 256
    return [(i * cn, cn) for i in range(n_ch)]


def _pad_count(maxc: int, dt: str) -> int:
    """Smallest padded token count for the dtype's matmul-rate constraints."""
    if dt == "f32r":
        # multiple of 256 >= maxc (min 512) so equal <=512 chunks stay >=256
        n = max(512, -(-maxc // 256) * 256)
        while n % (-(-n // 512)) != 0 or (n // (-(-n // 512))) % 2 != 0:
            n += 256
        return n
    # 16-bit: equal chunks of <=512, each even (4B DMA alignment)
    n_ch = -(-maxc // 512)
    cn = -(-maxc // n_ch)
    cn += cn % 2
    return n_ch * cn


def _legalize_sync_waits(nc):
    """Split sync waits exceeding the per-instruction ISA budget into NOPs.

    This walrus build rejects instructions with too many embedded sync-wait
    commands ("Too many sync wait commands", CoreV3GenImpl setupSyncWait):
    Matmult (fp32r, self-loading weights) tolerates 1, most opcodes 2, and
    Tile's scheduler freely emits more (e.g. the kernel-tail Drain). Moving
    the excess waits onto NoOp instructions placed immediately before the
    offender on the same engine queue is semantically identical: the engine
    blocks on the NOP first, then issues the original instruction.
    """
    ctr = 0
    for fn in nc.m.functions:
        for blk in fn.blocks:
            insts = blk.instructions
            out = []
            changed = False
            for inst in insts:
                si = inst.sync_info
                waits = list(si.on_wait) if si is not None and si.on_wait else []
                limit = 1
                if len(waits) > limit:
                    extra, keep = waits[:-limit], waits[-limit:]
                    for w in extra:
                        nop = mybir.InstNoOp(name=f"ant_sync_split_{ctr}", ins=[], outs=[])
                        ctr += 1
                        nop.engine = inst.engine
                        nop.sync_info = mybir.SyncInfo(on_wait=[w], on_update=[])
                        out.append(nop)
                    si.on_wait = keep
                    changed = True
                out.append(inst)
            if changed:
                blk.instructions = out
```
